# revision 31
# baseline (speedup 1.0000x reference)
"""Causal attention (B=4, L=2048, d_model=1024, d_k=d_v=128) on 8 TRN2 NeuronCores.

Sharding (SPMD -- one program, per-core data):
  core c -> batch b = c//2, parity par = c%2.
  Core handles q-blocks j = 2k+par for slot k in 0..7 (128 rows each).
  Each core receives ONLY its own parity's X^T columns (xq).  K/V for the
  other parity are NOT recomputed: each core projects K^T/V for its own
  1024 rows and the pair exchanges them with a 2-replica AllGather through
  DRAM bounce buffers (one for K^T, one for V, staggered so the K wire
  time hides under V's projection).  The readback picks the peer's shard
  with two predicated DMAs (cond = parity of partition_id); the skipped
  one still bumps its semaphore, so the instruction stream stays uniform.

Within a core (all matmuls contract on the partition dim):
  - Q^T/K^T are weight-stationary projections accumulating 8 d_model
    chunks in PSUM; X pieces stream in consumption order.
  - V is projected UN-transposed (X block stationary, W_V moving), so the
    AV matmul's rhs comes straight from the projection -- no PE
    transposes and no extra PSUM round trip.
  - Scores are computed TRANSPOSED: S^T[key, q] = K^T_blk.T @ Q^T.  The
    causal boundary mask is PRELOADED into PSUM (gpsimd copy) and the
    diagonal score matmul accumulates onto it with start=False -- nothing
    sits between the last score matmul and the exp.
  - One merged exp per (parity, key-slot) spanning all covered q-slots
    (up to 1024 wide) writes A^T straight to SBUF bf16.
  - V is augmented with a ones column; Z_aug = A^T.T @ [V | 1] yields the
    softmax denominator in column 128 for free.  Softmax skips the
    row-max subtraction (scores here are bounded ~|12|; exp is safe).
"""

import os
import sys

sys.path.insert(0, "/opt/trn_rl_repo")
sys.path.insert(0, "/opt/trn_rl_repo/concourse")

import ml_dtypes
import numpy as np

import concourse.bass as bass  # noqa: F401
import concourse.mybir as mybir
import concourse.tile as tile
from concourse import bacc
from concourse.bass_utils import run_bass_kernel_spmd
from concourse.masks import make_identity

B, L, DM, DK, DV = 4, 2048, 1024, 128, 128
NB = L // 128   # 16 key blocks per batch
SLOTS = 8       # q-blocks per core
NCH = DM // 128  # 8 d_model chunks
SCALE = float(DK) ** -0.5
MASKVAL = -1e9

NWARM = int(os.environ.get("ATTN_NWARM", "14"))
XBAR_VT = os.environ.get("ATTN_XBAR_VT", "0") == "1"

F32 = mybir.dt.float32
BF16 = mybir.dt.bfloat16
VA_W = SLOTS * (DV + 1)  # 1032

# X piece widths (columns of this core's 1024 X^T columns).  Small pieces
# so the in-order PE queue chases the DMA stream at fine granularity.
PIECES = [128] * 8
XO_PIECES = [256] * 4


def build_nc():
    nc = bacc.Bacc()

    # X^T, piece-contiguous: for each piece i the host stores [128, NCH, w_i]
    # with each partition row contiguous, so a piece DMA is 128 descriptors
    # of 2*NCH*w_i bytes instead of 128*NCH small ones (descriptor
    # generation on the Sync queue was gating the input stream).
    xq_ext = nc.declare_dram_parameter("xq", [128, NCH * SLOTS * 128], BF16,
                                       isOutput=False)
    xo_ext = nc.declare_dram_parameter("xo", [128, NCH * SLOTS * 128], BF16,
                                       isOutput=False)
    # weights pre-arranged on host: [p, c*128+d] = W[c*128+p, d]
    wq_ext = nc.declare_dram_parameter("wq", [128, DM], BF16, isOutput=False)
    wk_ext = nc.declare_dram_parameter("wk", [128, DM], BF16, isOutput=False)
    wv_ext = nc.declare_dram_parameter("wv", [128, DM], BF16, isOutput=False)
    # boundary masks, stored TRANSPOSED for the PE mask-writer matmul:
    # [q 128, 2*128 key] -- col block sp is M_sp^T where M_sp[key, q] is
    # added to the diagonal score block of parity sp (sp=0 own parity:
    # causal triangle; sp=1 other parity: all-masked on even cores /
    # all-zero on odd cores).  The mask enters PSUM via the PE
    # (maskT.T @ I, start=True) so the bank's zero-region marking and the
    # mask write form one deterministic in-order instruction stream --
    # an engine-side PSUM preload would race with the bank's start=True
    # matmul (start lazily zeroes the whole 2KB bank).
    mask_ext = nc.declare_dram_parameter("maskT", [128, 256], BF16, isOutput=False)
    out_ext = nc.declare_dram_parameter("out", [SLOTS * 128, DV], F32, isOutput=True)

    with tile.TileContext(nc) as tc:
        with (
            tc.tile_pool(name="persist", bufs=1) as persist,
            tc.tile_pool(name="st_ps", bufs=2, space="PSUM") as st_ps,
            tc.tile_pool(name="pp_ps", bufs=2, space="PSUM") as pp_ps,
            tc.tile_pool(name="z_ps", bufs=2, space="PSUM") as z_ps,
            tc.tile_pool(name="work", bufs=6) as work,
        ):
            # ---- PE warm-up: ramp DVFS while the first DMAs land ----
            ones = persist.tile([128, 128], BF16, tag="ones")
            nc.vector.memset(ones[:], 1.0)
            warm_ps = z_ps.tile([128, DV + 1], F32, tag="z", name="warm")
            for i in range(NWARM):
                nc.tensor.matmul(warm_ps[:, 0:128], ones[:], ones[:],
                                 start=(i == 0), stop=(i == NWARM - 1))

            # ---- inputs ----
            w_sb = {}

            def load_w(name, ext, eng):
                t = persist.tile([128, NCH, 128], BF16, tag=name, name=name)
                eng.dma_start(
                    out=t[:], in_=ext.rearrange("p (c d) -> p c d", d=128)
                )
                w_sb[name] = t

            def piece(ext, lo, w, nm, eng):
                t = persist.tile([128, NCH, w], BF16, tag=nm, name=nm)
                eng.dma_start(
                    out=t[:],
                    in_=ext[:, lo * NCH:(lo + w) * NCH].rearrange(
                        "p (c l) -> p c l", c=NCH),
                )
                return t

            # two HWDGE rings: xq pieces on the Sync queue, weights/xo/mask
            # on the Scalar queue so descriptor generation runs in parallel
            load_w("wk", wk_ext, nc.sync)
            load_w("wv", wv_ext, nc.sync)
            load_w("wq", wq_ext, nc.sync)
            mask_sb = persist.tile([128, 256], BF16, tag="mask")
            nc.sync.dma_start(out=mask_sb[:], in_=mask_ext[:])
            xq_p = []
            lo = 0
            for i, w in enumerate(PIECES):
                xq_p.append((piece(xq_ext, lo, w, f"xq{i}", nc.sync), lo, w))
                lo += w
            ident = persist.tile([128, 128], BF16, tag="ident")
            make_identity(nc, ident)
            xo_p = []
            lo = 0
            for i, w in enumerate(XO_PIECES):
                xo_p.append((piece(xo_ext, lo, w, f"xo{i}", nc.sync),
                             lo, w))
                lo += w

            # ---- persistent result tiles ----
            qt = [persist.tile([128, 512], BF16, tag=f"qt{g}", name=f"qt{g}")
                  for g in range(2)]
            # K^T per parity: [d_k 128, 1024 keys]
            kt = [persist.tile([128, SLOTS * 128], BF16, tag=f"kt{sp}",
                               name=f"kt{sp}") for sp in range(2)]
            # V (un-transposed, augmented): per parity one [128, 8*129] tile;
            # block m at cols [m*129, m*129+128], ones at col m*129+128
            va = [persist.tile([128, VA_W], BF16, tag=f"va{sp}", name=f"va{sp}")
                  for sp in range(2)]
            for sp in range(2):
                v3 = va[sp][:].rearrange("p (m x) -> p m x", x=DV + 1)
                nc.vector.memset(v3[:, :, DV:DV + 1], 1.0)
            # V^T staging per parity (projection output before transpose)
            vt = [persist.tile([128, SLOTS * 128], BF16, tag=f"vt{sp}",
                               name=f"vt{sp}") for sp in range(2)]
            # A^T tiles: [key 128, q 1024], cols m*128.. used
            at = {(sp, m): persist.tile([128, 1024], BF16, tag=f"at{sp}_{m}",
                                        name=f"at{sp}_{m}")
                  for sp in range(2) for m in range(SLOTS)}

            # ---- projections (all weight-stationary so LDWEIGHTS stays off
            # the critical path via fast-weight-load) ----
            def proj_piece(name, dst, p, base=0, copy_eng="vector"):
                # dst[128, cols] = W.T @ X (transposed out), one piece
                w = w_sb[name]
                t, lo, wd = p
                ps = pp_ps.tile([128, 512], F32, tag="pp", name=f"p{name}")
                for c in range(NCH):
                    nc.tensor.matmul(
                        ps[:, 0:wd], w[:, c, :], t[:, c, :],
                        start=(c == 0), stop=(c == NCH - 1),
                    )
                if copy_eng == "scalar":
                    nc.scalar.copy(dst[:, lo - base:lo - base + wd],
                                   ps[:, 0:wd])
                else:
                    nc.vector.tensor_copy(dst[:, lo - base:lo - base + wd],
                                          ps[:, 0:wd])

            def vtrans(sp, ms):
                # per-block PE transpose V^T -> V into the augmented AV tile
                v3 = va[sp][:].rearrange("p (m x) -> p m x", x=DV + 1)
                for m in ms:
                    vps = pp_ps.tile([128, 128], BF16, tag="pp", name="vtr")
                    nc.tensor.transpose(
                        vps[:], vt[sp][:, m * 128:(m + 1) * 128], ident[:])
                    nc.vector.tensor_copy(v3[:, m, 0:DV], vps[:])

            # ---- scores + exp for key-slot m of parity sp ----
            def scores(sp, ms):
                for m in ms:
                    st = st_ps.tile([128, 1024], F32, tag="st", name=f"s{sp}{m}")
                    kcol = kt[sp][:, m * 128:(m + 1) * 128]
                    g0 = m // 4
                    a = m - 4 * g0
                    # diag bank: mask-writer first (start=True zero-marks
                    # the bank and deposits the mask), then the rest and
                    # the diagonal product accumulate in PE issue order.
                    nc.tensor.matmul(
                        st[:, m * 128:(m + 1) * 128],
                        mask_sb[:, sp * 128:(sp + 1) * 128], ident[:],
                        start=True, stop=False, skip_group_check=True,
                    )
                    if a < 3:
                        nc.tensor.matmul(
                            st[:, (m + 1) * 128:(g0 + 1) * 512],
                            kcol, qt[g0][:, (a + 1) * 128:512],
                            start=False, stop=False, skip_group_check=True,
                        )
                    if g0 == 0:
                        nc.tensor.matmul(
                            st[:, 512:1024],
                            kcol, qt[1][:],
                            start=True, stop=True, skip_group_check=True,
                        )
                    nc.tensor.matmul(
                        st[:, m * 128:(m + 1) * 128],
                        kcol, qt[g0][:, a * 128:(a + 1) * 128],
                        start=False, stop=True, skip_group_check=True,
                    )
                    nc.scalar.activation(
                        at[(sp, m)][:, m * 128:1024],
                        st[:, m * 128:1024],
                        mybir.ActivationFunctionType.Exp,
                        bias=0.0, scale=1.0,
                    )

            # ---- A^T.T @ [V|1] for q-slot k ----
            def av(ks):
                for k in ks:
                    zp = z_ps.tile([128, DV + 1], F32, tag="z", name=f"z{k}")
                    # sp=0 terms first (their exps are long done), so the PE
                    # has work while the freshest sp=1 exp finishes
                    for sp in range(2):
                        for m in range(k + 1):
                            nc.tensor.matmul(
                                zp[:],
                                at[(sp, m)][:, k * 128:(k + 1) * 128],
                                va[sp][:, m * (DV + 1):(m + 1) * (DV + 1)],
                                start=(m == 0 and sp == 0),
                                stop=(m == k and sp == 1),
                            )
                    rcp = work.tile([128, 1], F32, tag="rcp")
                    nc.vector.reciprocal(rcp[:], zp[:, DV:DV + 1])
                    z_sb = work.tile([128, DV], F32, tag="zout")
                    nc.vector.tensor_scalar_mul(z_sb[:], zp[:, 0:DV], rcp[:])
                    nc.sync.dma_start(
                        out=out_ext[k * 128:(k + 1) * 128, :], in_=z_sb[:]
                    )

            # ---- emission order.  The PE queue executes IN ORDER, so this
            # is the execution schedule: chase each arriving X piece with
            # all three projections, then own-parity scores run while the
            # other parity's X lands, then the sp=1 scores/AV tail
            # interleaves so av(k) matmuls cover exp(1,k+1) latency. ----
            for p in xq_p:
                g = p[1] // 512
                proj_piece("wk", kt[0], p, copy_eng="scalar")
                proj_piece("wv", vt[0], p)
                proj_piece("wq", qt[g], p, base=g * 512, copy_eng="scalar")
            vtrans(0, range(SLOTS))
            scores(0, range(0, 8))
            for p in xo_p:
                proj_piece("wk", kt[1], p, copy_eng="scalar")
                proj_piece("wv", vt[1], p)
                vtrans(1, range(p[1] // 128, (p[1] + p[2]) // 128))
            scores(1, [0])
            for k in range(SLOTS):
                if k + 1 < SLOTS:
                    scores(1, [k + 1])
                av([k])

    nc.finalize()
    return nc


_NC = None


def _get_nc():
    global _NC
    if _NC is None:
        _NC = build_nc()
    return _NC


def _make_masks():
    p = np.arange(128)[:, None]   # key (partition)
    q = np.arange(128)[None, :]   # query (free)
    triT = np.where(p <= q, 0.0, MASKVAL).astype(np.float32)
    full = np.full((128, 128), MASKVAL, np.float32)
    zero = np.zeros((128, 128), np.float32)
    # col block 0: own-parity key-slot m == k (triangle, both core types);
    # col block 1: other-parity key-slot m == k (all-masked on even cores,
    # all-valid on odd cores).  Stored TRANSPOSED ([q, key]) for the PE
    # mask-writer (lhsT.T @ I puts M back in [key, q] orientation).
    def t(blocks):
        return np.concatenate([b.T for b in blocks], axis=1).astype(
            ml_dtypes.bfloat16)
    mask_even = t([triT, full])
    mask_odd = t([triT, zero])
    return mask_even, mask_odd


def kernel(X, W_Q, W_K, W_V):
    X = np.asarray(X, np.float32)
    W_Q = np.asarray(W_Q, np.float32) * SCALE
    W_K = np.asarray(W_K, np.float32)
    W_V = np.asarray(W_V, np.float32)

    nc = _get_nc()
    mask_even, mask_odd = _make_masks()

    def warr(W):
        return np.ascontiguousarray(
            W.astype(ml_dtypes.bfloat16).reshape(NCH, 128, DK)
            .transpose(1, 0, 2).reshape(128, NCH * DK)
        )

    wq = warr(W_Q)
    wk = warr(W_K)
    wv = warr(W_V)

    def xarr(xt_np, cols, pieces):
        # piece-contiguous layout: [128, sum_i NCH*w_i] where piece i holds
        # [p, c, l] with each partition row contiguous (one descriptor per
        # partition per piece DMA)
        xt = xt_np[:, cols].reshape(NCH, 128, SLOTS * 128)  # [c, p, l]
        parts = []
        lo = 0
        for w in pieces:
            parts.append(xt[:, :, lo:lo + w].transpose(1, 0, 2)
                         .reshape(128, NCH * w))
            lo += w
        return np.ascontiguousarray(np.concatenate(parts, axis=1))

    in_maps = []
    for c in range(8):
        b, par = c // 2, c % 2
        xt_np = np.ascontiguousarray(X[b].T).astype(ml_dtypes.bfloat16)
        qcols = np.concatenate(
            [np.arange((2 * k + par) * 128, (2 * k + par + 1) * 128)
             for k in range(SLOTS)]
        )
        m = {
            "xq": xarr(xt_np, qcols, PIECES),
            "wq": wq, "wk": wk, "wv": wv,
            "maskT": mask_odd if par else mask_even,
        }
        ocols = np.concatenate(
            [np.arange((2 * k + 1 - par) * 128, (2 * k + 2 - par) * 128)
             for k in range(SLOTS)]
        )
        m["xo"] = xarr(xt_np, ocols, XO_PIECES)
        in_maps.append(m)

    res = run_bass_kernel_spmd(nc, in_maps, list(range(8)))

    Z = np.zeros((B, L, DV), np.float32)
    for c in range(8):
        b, par = c // 2, c % 2
        o = res.results[c]["out"]
        for k in range(SLOTS):
            j = 2 * k + par
            Z[b, j * 128:(j + 1) * 128, :] = o[k * 128:(k + 1) * 128, :]
    return Z


# revision 32
# speedup vs baseline: 1.2190x; 1.2190x over previous
"""Causal attention (B=4, L=2048, d_model=1024, d_k=d_v=128) on 8 TRN2 NeuronCores.

Sharding (SPMD — one program, per-core data):
  core c -> batch b = c//2, parity par = c%2.
  Core handles q-blocks j = 2k+par for slot k in 0..7 (128 rows each).
  X^T's column blocks are split by parity into two slot-ordered inputs:
  xq (this core's query-parity blocks, which are also half the keys) and
  xo (the other parity's blocks).  Slot k attends key-slots 0..k of EACH
  parity — a uniform instruction stream across cores.  The causal
  boundary is uniform too: the diagonal (triangular) mask always lands on
  q-parity key-slot m == k, while other-parity key-slot m == k is fully
  masked (even cores) or fully valid (odd cores) — fed as mask data.
  Every core projects K/V for all 2048 rows of its batch (KV compute
  duplicated within a pair; no collectives).

Within a core (all matmuls contract on the partition dim):
  - Projections are weight-stationary per 512-column group, accumulating
    8 d_model chunks in PSUM; inputs stream in consumption order and each
    projection group chases its own DMA piece.
  - Scores are computed TRANSPOSED: S^T[key, q] = K^T_blk.T @ Q^T, one
    N<=512 matmul per (parity, key-slot, slot group of 4).  exp() then
    writes A^T straight to SBUF (bf16) — no PE transposes or copies for A.
  - V is augmented with a ones column; Z_aug = A^T.T @ [V | 1] yields the
    softmax denominator in column 128 for free.  Softmax skips the row-max
    subtraction (scores here are bounded ~|12|; exp is safe in f32).
"""

import os
import sys

sys.path.insert(0, "/opt/trn_rl_repo")
sys.path.insert(0, "/opt/trn_rl_repo/concourse")

import ml_dtypes
import numpy as np

import concourse.bass as bass  # noqa: F401
import concourse.mybir as mybir
import concourse.tile as tile
from concourse import bacc
from concourse.bass_utils import run_bass_kernel_spmd
from concourse.masks import make_identity

B, L, DM, DK, DV = 4, 2048, 1024, 128, 128
NB = L // 128   # 16 key blocks per batch
SLOTS = 8       # q-blocks per core
NCH = DM // 128  # 8 d_model chunks
SCALE = float(DK) ** -0.5
MASKVAL = -1e9

COMPUTE = os.environ.get("ATTN_COMPUTE", "bf16")  # "bf16" | "f32"

F32 = mybir.dt.float32


def _cdt():
    return mybir.dt.bfloat16 if COMPUTE == "bf16" else mybir.dt.float32


def _np_cdt():
    return ml_dtypes.bfloat16 if COMPUTE == "bf16" else np.float32


def build_nc():
    cdt = _cdt()
    nc = bacc.Bacc()

    # X^T columns split by parity, each slot-ordered: xq = this core's
    # query-parity blocks (also half the keys), xo = other-parity blocks
    xq_ext = nc.declare_dram_parameter("xq", [DM, SLOTS * 128], cdt, isOutput=False)
    xo_ext = nc.declare_dram_parameter("xo", [DM, SLOTS * 128], cdt, isOutput=False)
    # weights pre-arranged on host to the SBUF chunk layout
    # [p, c*128+d] = W[c*128+p, d] so the DMA is fully contiguous
    wq_ext = nc.declare_dram_parameter("wq", [128, DM], cdt, isOutput=False)
    wk_ext = nc.declare_dram_parameter("wk", [128, DM], cdt, isOutput=False)
    wv_ext = nc.declare_dram_parameter("wv", [128, DM], cdt, isOutput=False)
    # transposed boundary masks: [key 128, 2*128 q] — col block 0 applied at
    # key block 2k, col block 1 at key block 2k+1 (for slot k)
    mask_ext = nc.declare_dram_parameter("maskT", [128, 256], F32, isOutput=False)
    out_ext = nc.declare_dram_parameter("out", [SLOTS * 128, DV], F32, isOutput=True)

    with tile.TileContext(nc) as tc:
        with (
            tc.tile_pool(name="persist", bufs=1) as persist,
            tc.tile_pool(name="mm_ps", bufs=6, space="PSUM") as mm_ps,
            tc.tile_pool(name="z_ps", bufs=2, space="PSUM") as z_ps,
            tc.tile_pool(name="work", bufs=6) as work,
        ):
            # ---- constants / inputs ----
            ident = persist.tile([128, 128], cdt, tag="ident")
            make_identity(nc, ident)

            # PE warm-up: the tensor engine's DVFS ramps to full clock only
            # after ~3us of continuous execution, and the PE is otherwise
            # idle from ~7us (ident ready) to ~10us (first weights+X piece
            # landed).  Burn that window on dummy matmuls into a scratch
            # PSUM tile so the real projection stream starts at high clock.
            warm_ps = mm_ps.tile([128, 128], F32, tag="mm", name="warm")
            NWARM = 38
            for i in range(NWARM):
                nc.tensor.matmul(warm_ps[:], ident[:], ident[:],
                                 start=(i == 0), stop=(i == NWARM - 1))

            w_sb = {}

            def load_w(name, ext):
                t = persist.tile([128, NCH, 128], cdt, tag=name, name=name)
                nc.sync.dma_start(
                    out=t[:], in_=ext.rearrange("p (c d) -> p c d", d=128)
                )
                w_sb[name] = t

            # Every DMA gets its own tile sized to exactly one consumer's
            # need (dependency tracking is DMA-granular): 512-column pieces
            # spanning all 8 d_model chunks; projection group g chases
            # piece g.
            xq_r = xq_ext.rearrange("(c p) l -> p c l", p=128)
            xo_r = xo_ext.rearrange("(c p) l -> p c l", p=128)
            # single queue => ring order == issue order == consumption order
            def piece(r, lo, w, nm):
                t = persist.tile([128, NCH, w], cdt, tag=nm, name=nm)
                nc.sync.dma_start(out=t[:], in_=r[:, :, lo:lo + w])
                return t

            load_w("wq", wq_ext)
            # first 512 columns split in two so the PE can start after 0.5MB
            xq_a = piece(xq_r, 0, 256, "xqa")
            xq_b = piece(xq_r, 256, 256, "xqb")
            mask_sb = persist.tile([128, 256], F32, tag="mask")
            nc.sync.dma_start(out=mask_sb[:], in_=mask_ext[:])
            load_w("wk", wk_ext)
            load_w("wv", wv_ext)
            xq_c = piece(xq_r, 512, 512, "xqc")
            xo_a = piece(xo_r, 0, 512, "xoa")
            xo_b = piece(xo_r, 512, 512, "xob")
            # per projection group: list of (rhs-piece, psum column offset)
            xq_p = [[(xq_a, 0), (xq_b, 256)], [(xq_c, 0)]]
            xo_p = [[(xo_a, 0)], [(xo_b, 0)]]

            # ---- per-(parity s, group) tiles; s=0 query-parity, s=1 other
            qt = [persist.tile([128, 512], cdt, tag=f"qt{g}", name=f"qt{g}")
                  for g in range(2)]
            kt = {(sp, g): persist.tile([128, 512], cdt, tag=f"kt{sp}{g}",
                                        name=f"kt{sp}{g}")
                  for sp in range(2) for g in range(2)}
            vt = {(sp, g): persist.tile([128, 512], cdt, tag=f"vt{sp}{g}",
                                        name=f"vt{sp}{g}")
                  for sp in range(2) for g in range(2)}
            v_aug = {}
            for sp in range(2):
                for m in range(SLOTS):
                    t = persist.tile([128, DV + 1], cdt, tag=f"va{sp}{m}",
                                     name=f"va{sp}{m}")
                    nc.vector.memset(t[:, DV:DV + 1], 1.0)
                    v_aug[(sp, m)] = t
            at = {}
            for sp in range(2):
                for m in range(SLOTS):
                    for g in range(2):
                        if m <= 4 * g + 3:
                            at[(sp, m, g)] = persist.tile(
                                [128, 512], cdt, tag=f"at{sp}_{m}_{g}",
                                name=f"at{sp}_{m}_{g}")

            def proj(name, src, dst, scale, gs):
                w = w_sb[name]
                for g in gs:
                    for pi, (t, off) in enumerate(src[g]):
                        wd = t.shape[-1]
                        ps = mm_ps.tile([128, wd], F32, tag="mm",
                                        name=f"pj{g}_{pi}")
                        for c in range(NCH):
                            nc.tensor.matmul(
                                ps[:],
                                w[:, c, :],
                                t[:, c, :],
                                start=(c == 0),
                                stop=(c == NCH - 1),
                            )
                        dslice = dst[g][:, off:off + wd]
                        if scale is not None:
                            nc.scalar.activation(
                                dslice, ps[:],
                                mybir.ActivationFunctionType.Copy,
                                bias=0.0, scale=scale,
                            )
                        elif name == "wv":
                            # keep V^T copies off the Scalar engine (it owns
                            # the exps the V-transposes otherwise wait behind)
                            nc.vector.tensor_copy(dslice, ps[:])
                        else:
                            nc.scalar.copy(dslice, ps[:])

            # emission in stream-arrival order; the Tile scheduler
            # dispatches by readiness + this priority
            def vt_blocks(sp, ms):
                for m in ms:
                    vps = mm_ps.tile([128, 128], cdt, tag="mm", name="vps")
                    nc.tensor.transpose(
                        vps[:],
                        vt[(sp, m // 4)][:, (m % 4) * 128:(m % 4 + 1) * 128],
                        ident[:],
                    )
                    dst = v_aug[(sp, m)][:, 0:DV]
                    nc.vector.tensor_copy(dst, vps[:])

            def scores(sp, ms):
                # S^T for key-slot m of parity sp, covered by q-slots k >= m
                for m in ms:
                    for g in range(2):
                        lo = max(m, 4 * g)
                        if lo > 4 * g + 3:
                            continue
                        a = lo - 4 * g
                        st = mm_ps.tile([128, 512], F32, tag="mm")
                        nc.tensor.matmul(
                            st[:, a * 128:512],
                            kt[(sp, m // 4)][:, (m % 4) * 128:(m % 4 + 1) * 128],
                            qt[g][:, a * 128:512],
                            start=True, stop=True,
                            skip_group_check=True,
                        )
                        if 4 * g <= m <= 4 * g + 3:
                            # causal boundary: q-parity slot m gets the
                            # triangle, other-parity slot m is all-or-nothing
                            # by core parity (mask data)
                            qoff = (m - 4 * g) * 128
                            nc.vector.tensor_add(
                                st[:, qoff:qoff + 128],
                                st[:, qoff:qoff + 128],
                                mask_sb[:, sp * 128:(sp + 1) * 128],
                            )
                        nc.scalar.activation(
                            at[(sp, m, g)][:, a * 128:512],
                            st[:, a * 128:512],
                            mybir.ActivationFunctionType.Exp,
                            bias=0.0, scale=1.0,
                        )

            def av(ks):
                for k in ks:
                    g, q = k // 4, (k % 4) * 128
                    zp = z_ps.tile([128, DV + 1], F32, tag="z")
                    for m in range(k + 1):
                        for sp in range(2):
                            nc.tensor.matmul(
                                zp[:],
                                at[(sp, m, g)][:, q:q + 128],
                                v_aug[(sp, m)][:],
                                start=(m == 0 and sp == 0),
                                stop=(m == k and sp == 1),
                            )
                    rcp = work.tile([128, 1], F32, tag="rcp")
                    nc.vector.reciprocal(rcp[:], zp[:, DV:DV + 1])
                    z_sb = work.tile([128, DV], F32, tag="zout")
                    nc.vector.tensor_scalar_mul(z_sb[:], zp[:, 0:DV], rcp[:])
                    nc.sync.dma_start(
                        out=out_ext[k * 128:(k + 1) * 128, :], in_=z_sb[:]
                    )

            proj("wq", xq_p, qt, None, [0])
            proj("wk", xq_p, [kt[(0, 0)], kt[(0, 1)]], None, [0])
            proj("wv", xq_p, [vt[(0, 0)], vt[(0, 1)]], None, [0])
            proj("wq", xq_p, qt, None, [1])
            vt_blocks(0, range(0, 4))
            scores(0, range(0, 4))
            proj("wk", xq_p, [kt[(0, 0)], kt[(0, 1)]], None, [1])
            proj("wv", xq_p, [vt[(0, 0)], vt[(0, 1)]], None, [1])
            vt_blocks(0, range(4, 8))
            scores(0, range(4, 8))
            proj("wk", xo_p, [kt[(1, 0)], kt[(1, 1)]], None, [0])
            proj("wv", xo_p, [vt[(1, 0)], vt[(1, 1)]], None, [0])
            vt_blocks(1, range(0, 4))
            scores(1, range(0, 4))
            av(range(0, 4))
            proj("wk", xo_p, [kt[(1, 0)], kt[(1, 1)]], None, [1])
            proj("wv", xo_p, [vt[(1, 0)], vt[(1, 1)]], None, [1])
            vt_blocks(1, range(4, 8))
            scores(1, range(4, 8))
            av(range(4, 8))

    nc.finalize()
    return nc


_NC = None


def _get_nc():
    global _NC
    if _NC is None:
        _NC = build_nc()
    return _NC


def _make_masks():
    p = np.arange(128)[:, None]   # key (partition)
    q = np.arange(128)[None, :]   # query (free)
    triT = np.where(p <= q, 0.0, MASKVAL).astype(np.float32)
    full = np.full((128, 128), MASKVAL, np.float32)
    zero = np.zeros((128, 128), np.float32)
    # col block 0: q-parity key-slot m == k (diagonal, both parities);
    # col block 1: other-parity key-slot m == k (all-masked on even cores,
    # all-valid on odd cores)
    mask_even = np.concatenate([triT, full], axis=1)
    mask_odd = np.concatenate([triT, zero], axis=1)
    return mask_even, mask_odd


def kernel(X, W_Q, W_K, W_V):
    X = np.asarray(X, np.float32)
    W_Q = np.asarray(W_Q, np.float32) * SCALE
    W_K = np.asarray(W_K, np.float32)
    W_V = np.asarray(W_V, np.float32)

    nc = _get_nc()
    npdt = _np_cdt()
    mask_even, mask_odd = _make_masks()

    def warr(W):
        return np.ascontiguousarray(
            W.astype(npdt).reshape(NCH, 128, DK).transpose(1, 0, 2)
            .reshape(128, NCH * DK)
        )

    wq = warr(W_Q)
    wk = warr(W_K)
    wv = warr(W_V)

    in_maps = []
    for c in range(8):
        b, par = c // 2, c % 2
        xt_np = np.ascontiguousarray(X[b].T).astype(npdt)
        qcols = np.concatenate(
            [np.arange((2 * k + par) * 128, (2 * k + par + 1) * 128)
             for k in range(SLOTS)]
        )
        ocols = np.concatenate(
            [np.arange((2 * k + 1 - par) * 128, (2 * k + 2 - par) * 128)
             for k in range(SLOTS)]
        )
        in_maps.append({
            "xq": np.ascontiguousarray(xt_np[:, qcols]),
            "xo": np.ascontiguousarray(xt_np[:, ocols]),
            "wq": wq, "wk": wk, "wv": wv,
            "maskT": mask_odd if par else mask_even,
        })

    res = run_bass_kernel_spmd(nc, in_maps, list(range(8)))

    Z = np.zeros((B, L, DV), np.float32)
    for c in range(8):
        b, par = c // 2, c % 2
        o = res.results[c]["out"]
        for k in range(SLOTS):
            j = 2 * k + par
            Z[b, j * 128:(j + 1) * 128, :] = o[k * 128:(k + 1) * 128, :]
    return Z

